# revision 29
# baseline (speedup 1.0000x reference)
"""Trainium2 Bass kernel for nn_AreaEmbedding (masked triplet hinge loss).

Math (reference):
    loss = hier + sum_{i,j,k} [pos(i,j) & neg(i,k)] * relu(D2[i,j] - D2[i,k] + a)
    pos(i,j) = (j in x[i]) & (j != i);  neg(i,k) = (k not in x[i]) & (k != i)
    D2[i,j] = ||y_i - y_j||^2
    hier = ||wid-ken||^2 + ||wid-lrg||^2 + ||lrg-sml||^2 + ||sml-yad||^2

Key algebraic restructuring:
    pos has at most K=16 entries per row -> enumerate positives directly:
      sum_i sum_{jslot<16} wbase[i,js] * sum_k relu(dpos[i,js] - E[i,k])
    with
      dpos[i,js] = ||ypos - y_i||^2        (bias of the hinge instruction)
      E[i,k] = sq_k - 2<y_i,y_k> + sq_i + BIG*[k in x[i] or k==i] - alpha
    (sq_i is folded into E so the bias needs no correction; alpha and the
    neg-mask penalty are folded into the host-built pen tensor).
    wbase de-duplicates repeated x values and drops j == i; it is applied on
    the host to the per-slot row sums (pure masked summation).

Sharding: i-axis slabs of 64 rows per core across 8 NeuronCores.
Per-core partition packing: p = local_i + 64*h, where h selects the k-half
[h*256,(h+1)*256).  Bias column s covers jslot h*8+s for s<8 (read from
dpos=praw) and jslot (1-h)*8+(s-8) for s>=8 (partner partitions' values,
exchanged via a permutation matmul and copied to SBUF).  Every (jslot, k)
pair is covered exactly once.

Engines: TensorE computes E in bf16 (matmul -2*Yslab^T @ Y^T plus an
all-ones stationary times Y^2 for sq_k, accumulated in PSUM).  The 16 hinge
row-sum instructions are split ScalarE/VectorE (7/9): ScalarE
activation(Relu, scale=-1, bias, accum_out) and VectorE
scalar_tensor_tensor (E-c) min 0 with summed accum.  (GpSimd elementwise
measured ~8x slower than DVE here, so it only does the hier subtract and
small memsets/DMAs.)  Row sums are DMA'd out per engine group on separate
queues; the host applies the 0/1 dedup mask and signs.
"""

import os

import numpy as np

N, D, K = 512, 128, 16
NCORES = 8
NI = N // NCORES  # 64 rows per core
ALPHA = 0.1
BIG = 1.0e6
KH = 256  # k-half width

ACT_SLOTS = list(range(0, 8))    # relu-form, sign +1
DVE_SLOTS = list(range(8, 16))   # min-form, sign -1
GPS_SLOTS = []                   # gpsimd elementwise is ~10x slower; unused

LAST_EXEC_TIME_NS = None
_NC_CACHE = {}


def _bf16(a):
    import ml_dtypes

    return np.asarray(a, dtype=np.float32).astype(ml_dtypes.bfloat16)


def _jslot(h, s):
    """Bias column s on a partition in half h refers to this jslot."""
    return h * 8 + s if s < 8 else (1 - h) * 8 + (s - 8)


def _wbase(x):
    """[N, K] 0/1: first occurrence of value in row, and value != row index."""
    n, k = x.shape
    w = np.zeros((n, k), np.float32)
    for i in range(n):
        seen = set()
        for s in range(k):
            v = int(x[i, s])
            if v != i and v not in seen:
                w[i, s] = 1.0
            seen.add(v)
    return w


def _slot_weights(wbase_sl):
    """[128, 16] mask: w[p, s] = wbase[i(p), jslot(h(p), s)] (no signs)."""
    w = np.zeros((128, 16), np.float32)
    for s in range(16):
        for h in (0, 1):
            w[h * 64 : (h + 1) * 64, s] = wbase_sl[:, _jslot(h, s)]
    return w


def _host_pack(yad, wid, ken, lrg, sml, x):
    """Build the 8 per-core input dicts (indexing / mask construction only)."""
    yadT = np.ascontiguousarray(yad.T)  # [128, 512]
    wbase = _wbase(x)
    perm = np.zeros((128, 128), np.float32)
    for m in range(128):
        perm[(m + 64) % 128, m] = 1.0

    yt_bf = _bf16(yadT)
    in_maps = []
    weights = []
    for c in range(NCORES):
        i0 = c * NI
        sl = slice(i0, i0 + NI)
        xi = x[sl]  # [64, 16]

        yslabt = np.ascontiguousarray(yad[sl].T)  # [128, 64]

        # penalty [128, 256] minus alpha: p = li + 64*h covers k-half h
        rows = np.repeat(np.arange(NI), K)
        cols = xi.reshape(-1)
        mask = np.zeros((NI, N), np.float32)
        mask[rows, cols] = BIG
        mask[np.arange(NI), np.arange(NI) + i0] = BIG
        pen = np.empty((128, KH), np.float32)
        pen[0:64] = mask[:, 0:KH]
        pen[64:128] = mask[:, KH:]
        pen -= ALPHA

        # ypos [128, 8, 128]: slot s on (li, h) is jslot h*8+s
        ypos = np.empty((128, 8, D), np.float32)
        ypos[0:64] = yad[xi[:, 0:8]]
        ypos[64:128] = yad[xi[:, 8:16]]

        # ys2rep [128, 4, 128]: y_i replicated (4 slots; reused for both halves)
        ys2 = np.concatenate([yad[sl], yad[sl]], axis=0)  # [128, 128]
        ys2rep = np.broadcast_to(ys2[:, None, :], (128, 4, D))

        # hier stacked + packed to [128, 256]
        ha = np.concatenate([wid[sl], wid[sl], lrg[sl], sml[sl]], axis=1)
        hb = np.concatenate([ken[sl], lrg[sl], sml[sl], yad[sl]], axis=1)
        ha128 = np.concatenate([ha[:, 0:256], ha[:, 256:512]], axis=0)
        hb128 = np.concatenate([hb[:, 0:256], hb[:, 256:512]], axis=0)

        in_maps.append(
            {
                "yt": yt_bf,
                "yslabt": _bf16(yslabt),
                "pen": _bf16(pen),
                "ypos": _bf16(ypos.reshape(128, 8 * D)),
                "ys2rep": _bf16(ys2rep.reshape(128, 4 * D)),
                "permt": _bf16(perm),
                "ha": _bf16(ha128),
                "hb": _bf16(hb128),
            }
        )
        weights.append(_slot_weights(wbase[sl]))
    return in_maps, weights


def _gather_host(results, weights):
    """Mask-weighted reduction of the per-core device partials (float64)."""
    total = 0.0
    for r, w in zip(results, weights):
        oa = r["outa"].astype(np.float64)  # [128, 9]
        od = r["outd"].astype(np.float64)  # [128, 9]
        w = w.astype(np.float64)
        for ci, s in enumerate(ACT_SLOTS):
            total += (w[:, s] * oa[:, ci]).sum()
        total += oa[:, 8].sum()  # hier
        for ci, s in enumerate(DVE_SLOTS):
            total -= (w[:, s] * od[:, ci]).sum()
    return total


def model_numpy(packed):
    """Numpy emulation of the device algorithm (f32; layouts mirrored)."""
    in_maps, weights = packed
    results = []
    for m in in_maps:
        yt = m["yt"].astype(np.float64)  # [128, 512]
        yslabt = m["yslabt"].astype(np.float64)  # [128, 64]
        pen = m["pen"].astype(np.float64)
        ypos = m["ypos"].astype(np.float64).reshape(128, 8, D)
        ys2rep = m["ys2rep"].astype(np.float64).reshape(128, 4, D)
        permt = m["permt"].astype(np.float64)

        sq = (yt * yt).sum(axis=0)  # [512]
        sqi = (ys2rep[:, 0, :] ** 2).sum(axis=-1)  # [128]
        g = yslabt.T @ yt  # [64, 512]
        e = np.empty((128, KH))
        for h in (0, 1):
            e[h * 64 : (h + 1) * 64] = (
                -2.0 * g[:, h * KH : (h + 1) * KH] + sq[None, h * KH : (h + 1) * KH]
            )
        e = e + pen + sqi[:, None]

        diff = ypos - np.concatenate([ys2rep, ys2rep], axis=1)
        praw = (diff * diff).sum(axis=-1)  # [128, 8] = dpos
        prawsw = permt.T @ praw
        c_full = np.concatenate([praw, prawsw], axis=1)  # [128, 16]

        oa = np.zeros((128, 9))
        od = np.zeros((128, 9))
        for ci, s in enumerate(ACT_SLOTS):
            oa[:, ci] = np.maximum(c_full[:, s : s + 1] - e, 0.0).sum(axis=1)
        dh = m["ha"].astype(np.float64) - m["hb"].astype(np.float64)
        oa[:, 8] = (dh * dh).sum(axis=1)
        for ci, s in enumerate(DVE_SLOTS):
            od[:, ci] = np.minimum(e - c_full[:, s : s + 1], 0.0).sum(axis=1)
        results.append({"outa": oa, "outd": od})
    return _gather_host(results, weights)


def _build_nc():
    import concourse.tile as tile
    from concourse import bacc, mybir

    f32 = mybir.dt.float32
    bf16 = mybir.dt.bfloat16
    nc = bacc.Bacc("TRN2", target_bir_lowering=False)

    yt_d = nc.dram_tensor("yt", [128, 512], bf16, kind="ExternalInput")
    yslabt_d = nc.dram_tensor("yslabt", [128, 64], bf16, kind="ExternalInput")
    pen_d = nc.dram_tensor("pen", [128, KH], bf16, kind="ExternalInput")
    ypos_d = nc.dram_tensor("ypos", [128, 8 * D], bf16, kind="ExternalInput")
    ys2rep_d = nc.dram_tensor("ys2rep", [128, 4 * D], bf16, kind="ExternalInput")
    permt_d = nc.dram_tensor("permt", [128, 128], bf16, kind="ExternalInput")
    ha_d = nc.dram_tensor("ha", [128, KH], bf16, kind="ExternalInput")
    hb_d = nc.dram_tensor("hb", [128, KH], bf16, kind="ExternalInput")
    outa_d = nc.dram_tensor("outa", [128, 9], f32, kind="ExternalOutput")
    outd_d = nc.dram_tensor("outd", [128, 9], f32, kind="ExternalOutput")

    with tile.TileContext(nc) as tc:
        with (
            tc.tile_pool(name="io", bufs=1) as io,
            tc.tile_pool(name="wk", bufs=1) as wk,
            tc.tile_pool(name="psum", bufs=1, space="PSUM") as psum,
        ):
            yt = io.tile([128, 512], bf16)
            yslabt = io.tile([128, 64], bf16)
            pen = io.tile([128, KH], bf16)
            ypos = io.tile([128, 8, D], bf16)
            ys2rep = io.tile([128, 4, D], bf16)
            permt = io.tile([128, 128], bf16)
            ha = io.tile([128, KH], bf16)
            hb = io.tile([128, KH], bf16)

            # DMA queues (completion latency ~4us; firsts matter):
            nc.sync.dma_start(out=ypos[:], in_=ypos_d[:].rearrange("p (s d) -> p s d", s=8))
            nc.sync.dma_start(out=pen[:], in_=pen_d[:])
            nc.scalar.dma_start(out=ys2rep[:], in_=ys2rep_d[:].rearrange("p (s d) -> p s d", s=4))
            nc.scalar.dma_start(out=ha[:], in_=ha_d[:])
            nc.scalar.dma_start(out=hb[:], in_=hb_d[:])
            nc.gpsimd.dma_start(out=yt[:], in_=yt_d[:])
            nc.gpsimd.dma_start(out=yslabt[:], in_=yslabt_d[:])
            nc.gpsimd.dma_start(out=permt[:], in_=permt_d[:])

            ones = wk.tile([128, 64], bf16)
            nc.gpsimd.memset(ones[:], 1.0)
            zeros = wk.tile([128, KH], bf16)
            nc.gpsimd.memset(zeros[:], 0.0)

            # ---------------- c-path: praw[p,s] = ||ypos - y||^2 (VectorE)
            diff = wk.tile([128, 8, D], bf16)
            nc.vector.tensor_sub(diff[:, 0:4, :], ypos[:, 0:4, :], ys2rep[:])
            nc.vector.tensor_sub(diff[:, 4:8, :], ypos[:, 4:8, :], ys2rep[:])
            dsq = wk.tile([128, 8, D], bf16)
            nc.vector.tensor_mul(dsq[:], diff[:], diff[:])
            praw = wk.tile([128, 8], f32)

            # ---------------- E path
            sqi = wk.tile([128, 1], f32)
            scr_q = wk.tile([128, D], bf16)
            nc.scalar.activation(
                out=scr_q[:], in_=ys2rep[:, 0, :],
                func=mybir.ActivationFunctionType.Square, accum_out=sqi[:],
            )
            ytsq = wk.tile([128, 512], bf16)
            nc.scalar.activation(
                out=ytsq[:], in_=yt[:], func=mybir.ActivationFunctionType.Square
            )
            n2yst = wk.tile([128, 64], bf16)
            nc.vector.tensor_scalar_mul(n2yst[:], yslabt[:], -2.0)

            psum_e = psum.tile([128, KH], f32)
            for h in (0, 1):
                pslice = psum_e[h * 64 : (h + 1) * 64, :]
                ksl = slice(h * KH, (h + 1) * KH)
                tp = (0, h * 64)
                nc.tensor.matmul(
                    pslice, n2yst[:], yt[:, ksl], start=True, stop=False,
                    tile_position=tp,
                )
                nc.tensor.matmul(
                    pslice, ones[:], ytsq[:, ksl], start=False, stop=True,
                    tile_position=tp,
                )

            # E = psum_e + sqi + pen  (one fused stt on VectorE), then praw
            e_sb = wk.tile([128, KH], bf16)
            nc.vector.scalar_tensor_tensor(
                out=e_sb[:], in0=psum_e[:], scalar=sqi[:], in1=pen[:],
                op0=mybir.AluOpType.add, op1=mybir.AluOpType.add,
            )
            praw_bf = wk.tile([128, 8], bf16)
            psum_p = psum.tile([128, 8], f32)
            nc.vector.reduce_sum(praw[:, 0:4], dsq[:, 0:4, :], axis=mybir.AxisListType.X)
            nc.vector.tensor_copy(praw_bf[:, 0:4], praw[:, 0:4])
            nc.tensor.matmul(psum_p[:, 0:4], permt[:], praw_bf[:, 0:4], start=True, stop=True)
            nc.vector.reduce_sum(praw[:, 4:8], dsq[:, 4:8, :], axis=mybir.AxisListType.X)
            nc.vector.tensor_copy(praw_bf[:, 4:8], praw[:, 4:8])
            nc.tensor.matmul(psum_p[:, 4:8], permt[:], praw_bf[:, 4:8], start=True, stop=True)

            # ---------------- hier (early, on ScalarE + GpSimd)
            dh = wk.tile([128, KH], bf16)
            nc.gpsimd.tensor_sub(dh[:], ha[:], hb[:])
            outa = wk.tile([128, 9], f32)
            outd = wk.tile([128, 9], f32)
            rs_ps = psum.tile([128, 9], f32)
            scr_h = wk.tile([128, KH], bf16)
            nc.scalar.activation(
                out=scr_h[:], in_=dh[:],
                func=mybir.ActivationFunctionType.Square,
                accum_out=rs_ps[:, 8:9],
            )

            # ---------------- 16 hinge row-sum instructions
            scr_a = wk.tile([128, KH], bf16)
            scr_d = wk.tile([128, KH], bf16)

            def bias(s):
                return praw[:, s : s + 1] if s < 8 else psum_p[:, s - 8 : s - 7]

            for ci, s in enumerate(ACT_SLOTS):
                nc.scalar.activation(
                    out=scr_a[:], in_=e_sb[:],
                    func=mybir.ActivationFunctionType.Relu,
                    bias=bias(s), scale=-1.0,
                    accum_out=rs_ps[:, ci : ci + 1],
                )
            dve_order = [8, 9, 10, 11, 12, 13, 14, 15]
            for s in dve_order:
                ci = DVE_SLOTS.index(s)
                nc.vector.scalar_tensor_tensor(
                    out=scr_d[:], in0=e_sb[:], scalar=bias(s), in1=zeros[:],
                    op0=mybir.AluOpType.subtract, op1=mybir.AluOpType.min,
                    accum_out=outd[:, ci : ci + 1],
                )
            nc.scalar.copy(outa[:], rs_ps[:])
            nc.gpsimd.dma_start(out=outa_d[:], in_=outa[:])
            nc.sync.dma_start(out=outd_d[:], in_=outd[:])

    nc.finalize()
    return nc


def _get_nc():
    if "nc" not in _NC_CACHE:
        _NC_CACHE["nc"] = _build_nc()
    return _NC_CACHE["nc"]


def _install_ntff_hook():
    """Provide antenv.axon_hooks if the image lacks it, so trace=True can
    capture NTFF profiles through the axon PJRT .so."""
    import sys
    import types

    try:
        from antenv.axon_hooks import get_axon_ntff_profile_hook  # noqa: F401

        return
    except ImportError:
        pass
    try:
        import antenv
        from trn_agent_boot.trn_boot import _ntff_profile_via_ctypes
    except ImportError:
        return
    mod = types.ModuleType("antenv.axon_hooks")
    state = {"h": None}
    mod.set_axon_ntff_profile_hook = lambda h: state.__setitem__("h", h)
    mod.get_axon_ntff_profile_hook = lambda: state["h"]
    sys.modules["antenv.axon_hooks"] = mod
    antenv.axon_hooks = mod
    try:
        hook = _ntff_profile_via_ctypes("/opt/axon/libaxon_pjrt.so")
    except OSError:
        hook = None
    mod.set_axon_ntff_profile_hook(hook)


def kernel(wid_pos_mu, ken_pos_mu, lrg_pos_mu, sml_pos_mu, yad_pos, x):
    global LAST_EXEC_TIME_NS
    wid = np.asarray(wid_pos_mu, dtype=np.float32)
    ken = np.asarray(ken_pos_mu, dtype=np.float32)
    lrg = np.asarray(lrg_pos_mu, dtype=np.float32)
    sml = np.asarray(sml_pos_mu, dtype=np.float32)
    yad = np.asarray(yad_pos, dtype=np.float32)
    xi = np.asarray(x).astype(np.int64)

    in_maps, weights = _host_pack(yad, wid, ken, lrg, sml, xi)

    from concourse.bass_utils import run_bass_kernel_spmd

    nc = _get_nc()
    trace = bool(int(os.environ.get("KERNEL_TRACE", "0")))
    if trace:
        _install_ntff_hook()
    res = run_bass_kernel_spmd(
        nc, in_maps, core_ids=list(range(NCORES)), trace=trace,
        tmpdir=os.environ.get("KERNEL_TMPDIR") or None,
    )
    LAST_EXEC_TIME_NS = res.exec_time_ns

    return np.float32(_gather_host(res.results, weights))


if __name__ == "__main__":
    # Smoke test of the numpy model against a direct dense recompute.
    rng = np.random.default_rng(0)
    yad = rng.standard_normal((N, D)).astype(np.float32)
    wid = rng.standard_normal((N, D)).astype(np.float32)
    ken = rng.standard_normal((N, D)).astype(np.float32)
    lrg = rng.standard_normal((N, D)).astype(np.float32)
    sml = rng.standard_normal((N, D)).astype(np.float32)
    x = rng.integers(0, N, size=(N, K)).astype(np.int64)

    def dense_ref(wid, ken, lrg, sml, yad, x):
        loss = (
            ((wid - ken) ** 2).sum()
            + ((wid - lrg) ** 2).sum()
            + ((lrg - sml) ** 2).sum()
            + ((sml - yad) ** 2).sum()
        )
        m = np.zeros((N, N), bool)
        m[np.arange(N)[:, None], x] = True
        eye = np.eye(N, dtype=bool)
        pos = m & ~eye
        neg = (~m) & ~eye
        sq = (yad * yad).sum(-1)
        gram = yad @ yad.T
        d2 = sq[:, None] + sq[None, :] - 2.0 * gram
        t = d2[:, :, None] - d2[:, None, :] + ALPHA
        valid = pos[:, :, None] & neg[:, None, :]
        return loss + np.where(valid, np.maximum(t, 0.0), 0.0).sum()

    ref = dense_ref(
        wid.astype(np.float64), ken.astype(np.float64), lrg.astype(np.float64),
        sml.astype(np.float64), yad.astype(np.float64), x,
    )
    got = model_numpy(_host_pack(yad, wid, ken, lrg, sml, x))
    print("dense ref:", ref)
    print("model    :", got)
    print("rel err  :", abs(got - ref) / abs(ref))


# revision 30
# speedup vs baseline: 1.1557x; 1.1557x over previous
"""Trainium2 Bass kernel for nn_AreaEmbedding (masked triplet hinge loss).

Math (reference):
    loss = hier + sum_{i,j,k} [pos(i,j) & neg(i,k)] * relu(D2[i,j] - D2[i,k] + a)
    pos(i,j) = (j in x[i]) & (j != i);  neg(i,k) = (k not in x[i]) & (k != i)
    D2[i,j] = ||y_i - y_j||^2
    hier = ||wid-ken||^2 + ||wid-lrg||^2 + ||lrg-sml||^2 + ||sml-yad||^2

Key algebraic restructuring:
    pos has at most K=16 entries per row -> enumerate positives directly:
      sum_i sum_{jslot<16} wbase[i,js] * sum_k relu(dpos[i,js] - E[i,k])
    with
      dpos[i,js] = ||ypos - y_i||^2        (bias of the hinge instruction)
      E[i,k] = sq_k - 2<y_i,y_k> + sq_i + BIG*[k in x[i] or k==i] - alpha
    (sq_i is folded into E so the bias needs no correction; alpha and the
    neg-mask penalty are folded into the host-built pen tensor).
    wbase de-duplicates repeated x values and drops j == i; it is applied on
    the host to the per-slot row sums (pure masked summation).

Sharding: i-axis slabs of 64 rows per core across 8 NeuronCores.
Per-core partition packing: p = local_i + 64*h, where h selects the k-half
[h*256,(h+1)*256).  Bias column s covers jslot h*8+s for s<8 (read from
dpos=praw) and jslot (1-h)*8+(s-8) for s>=8 (partner partitions' values,
exchanged via a permutation matmul and copied to SBUF).  Every (jslot, k)
pair is covered exactly once.

Engines: TensorE computes E in bf16 (matmul -2*Yslab^T @ Y^T plus an
all-ones stationary times Y^2 for sq_k, accumulated in PSUM).  The 16 hinge
row-sum instructions are split ScalarE/VectorE (7/9): ScalarE
activation(Relu, scale=-1, bias, accum_out) and VectorE
scalar_tensor_tensor (E-c) min 0 with summed accum.  (GpSimd elementwise
measured ~8x slower than DVE here, so it only does the hier subtract and
small memsets/DMAs.)  Row sums are DMA'd out per engine group on separate
queues; the host applies the 0/1 dedup mask and signs.
"""

import os

import numpy as np

N, D, K = 512, 128, 16
NCORES = 8
NI = N // NCORES  # 64 rows per core
ALPHA = 0.1
BIG = 1.0e6
KH = 256  # k-half width

ACT_SLOTS = list(range(0, 8))    # relu-form, sign +1
DVE_SLOTS = list(range(8, 16))   # min-form, sign -1
GPS_SLOTS = []                   # gpsimd elementwise is ~10x slower; unused

LAST_EXEC_TIME_NS = None
_NC_CACHE = {}


def _bf16(a):
    import ml_dtypes

    return np.asarray(a, dtype=np.float32).astype(ml_dtypes.bfloat16)


def _jslot(h, s):
    """Bias column s on a partition in half h refers to this jslot."""
    return h * 8 + s if s < 8 else (1 - h) * 8 + (s - 8)


def _wbase(x):
    """[N, K] 0/1: first occurrence of value in row, and value != row index."""
    n, k = x.shape
    w = np.zeros((n, k), np.float32)
    for i in range(n):
        seen = set()
        for s in range(k):
            v = int(x[i, s])
            if v != i and v not in seen:
                w[i, s] = 1.0
            seen.add(v)
    return w


def _slot_weights(wbase_sl):
    """[128, 16] mask: w[p, s] = wbase[i(p), jslot(h(p), s)] (no signs)."""
    w = np.zeros((128, 16), np.float32)
    for s in range(16):
        for h in (0, 1):
            w[h * 64 : (h + 1) * 64, s] = wbase_sl[:, _jslot(h, s)]
    return w


def _host_pack(yad, wid, ken, lrg, sml, x):
    """Build the 8 per-core input dicts (indexing / mask construction only)."""
    yadT = np.ascontiguousarray(yad.T)  # [128, 512]
    wbase = _wbase(x)
    perm = np.zeros((128, 128), np.float32)
    for m in range(128):
        perm[(m + 64) % 128, m] = 1.0

    yt_bf = _bf16(yadT)
    in_maps = []
    weights = []
    for c in range(NCORES):
        i0 = c * NI
        sl = slice(i0, i0 + NI)
        xi = x[sl]  # [64, 16]

        yslabt = np.ascontiguousarray(yad[sl].T)  # [128, 64]

        # penalty [128, 256] minus alpha: p = li + 64*h covers k-half h
        rows = np.repeat(np.arange(NI), K)
        cols = xi.reshape(-1)
        mask = np.zeros((NI, N), np.float32)
        mask[rows, cols] = BIG
        mask[np.arange(NI), np.arange(NI) + i0] = BIG
        pen = np.empty((128, KH), np.float32)
        pen[0:64] = mask[:, 0:KH]
        pen[64:128] = mask[:, KH:]
        pen -= ALPHA

        # ypos [128, 8, 128]: slot s on (li, h) is jslot h*8+s
        ypos = np.empty((128, 8, D), np.float32)
        ypos[0:64] = yad[xi[:, 0:8]]
        ypos[64:128] = yad[xi[:, 8:16]]

        # ys2rep [128, 4, 128]: y_i replicated (4 slots; reused for both halves)
        ys2 = np.concatenate([yad[sl], yad[sl]], axis=0)  # [128, 128]
        ys2rep = np.broadcast_to(ys2[:, None, :], (128, 4, D))

        # hier stacked + packed to [128, 256]
        ha = np.concatenate([wid[sl], wid[sl], lrg[sl], sml[sl]], axis=1)
        hb = np.concatenate([ken[sl], lrg[sl], sml[sl], yad[sl]], axis=1)
        ha128 = np.concatenate([ha[:, 0:256], ha[:, 256:512]], axis=0)
        hb128 = np.concatenate([hb[:, 0:256], hb[:, 256:512]], axis=0)

        in_maps.append(
            {
                "yt": yt_bf,
                "yslabt": _bf16(yslabt),
                "pen": _bf16(pen),
                "ypos": _bf16(ypos.reshape(128, 8 * D)),
                "ys2rep": _bf16(ys2rep.reshape(128, 4 * D)),
                "permt": _bf16(perm),
                "ha": _bf16(ha128),
                "hb": _bf16(hb128),
            }
        )
        weights.append(_slot_weights(wbase[sl]))
    return in_maps, weights


def _gather_host(results, weights):
    """Mask-weighted reduction of the per-core device partials (float64)."""
    total = 0.0
    for r, w in zip(results, weights):
        oa = r["outa"].astype(np.float64)  # [128, 9]
        od = r["outd"].astype(np.float64)  # [128, 9]
        w = w.astype(np.float64)
        for ci, s in enumerate(ACT_SLOTS):
            total += (w[:, s] * oa[:, ci]).sum()
        total += oa[:, 8].sum()  # hier
        for ci, s in enumerate(DVE_SLOTS):
            total -= (w[:, s] * od[:, ci]).sum()
    return total


def model_numpy(packed):
    """Numpy emulation of the device algorithm (f32; layouts mirrored)."""
    in_maps, weights = packed
    results = []
    for m in in_maps:
        yt = m["yt"].astype(np.float64)  # [128, 512]
        yslabt = m["yslabt"].astype(np.float64)  # [128, 64]
        pen = m["pen"].astype(np.float64)
        ypos = m["ypos"].astype(np.float64).reshape(128, 8, D)
        ys2rep = m["ys2rep"].astype(np.float64).reshape(128, 4, D)
        permt = m["permt"].astype(np.float64)

        sq = (yt * yt).sum(axis=0)  # [512]
        sqi = (ys2rep[:, 0, :] ** 2).sum(axis=-1)  # [128]
        g = yslabt.T @ yt  # [64, 512]
        e = np.empty((128, KH))
        for h in (0, 1):
            e[h * 64 : (h + 1) * 64] = (
                -2.0 * g[:, h * KH : (h + 1) * KH] + sq[None, h * KH : (h + 1) * KH]
            )
        e = e + pen + sqi[:, None]

        diff = ypos - np.concatenate([ys2rep, ys2rep], axis=1)
        praw = (diff * diff).sum(axis=-1)  # [128, 8] = dpos
        prawsw = permt.T @ praw
        c_full = np.concatenate([praw, prawsw], axis=1)  # [128, 16]

        oa = np.zeros((128, 9))
        od = np.zeros((128, 9))
        for ci, s in enumerate(ACT_SLOTS):
            oa[:, ci] = np.maximum(c_full[:, s : s + 1] - e, 0.0).sum(axis=1)
        dh = m["ha"].astype(np.float64) - m["hb"].astype(np.float64)
        oa[:, 8] = (dh * dh).sum(axis=1)
        for ci, s in enumerate(DVE_SLOTS):
            od[:, ci] = np.minimum(e - c_full[:, s : s + 1], 0.0).sum(axis=1)
        results.append({"outa": oa, "outd": od})
    return _gather_host(results, weights)


def _build_nc():
    import concourse.tile as tile
    from concourse import bacc, mybir

    f32 = mybir.dt.float32
    bf16 = mybir.dt.bfloat16
    nc = bacc.Bacc("TRN2", target_bir_lowering=False)

    yt_d = nc.dram_tensor("yt", [128, 512], bf16, kind="ExternalInput")
    yslabt_d = nc.dram_tensor("yslabt", [128, 64], bf16, kind="ExternalInput")
    pen_d = nc.dram_tensor("pen", [128, KH], bf16, kind="ExternalInput")
    ypos_d = nc.dram_tensor("ypos", [128, 8 * D], bf16, kind="ExternalInput")
    ys2rep_d = nc.dram_tensor("ys2rep", [128, 4 * D], bf16, kind="ExternalInput")
    permt_d = nc.dram_tensor("permt", [128, 128], bf16, kind="ExternalInput")
    ha_d = nc.dram_tensor("ha", [128, KH], bf16, kind="ExternalInput")
    hb_d = nc.dram_tensor("hb", [128, KH], bf16, kind="ExternalInput")
    outa_d = nc.dram_tensor("outa", [128, 9], f32, kind="ExternalOutput")
    outd_d = nc.dram_tensor("outd", [128, 9], f32, kind="ExternalOutput")

    with tile.TileContext(nc) as tc:
        with (
            tc.tile_pool(name="io", bufs=1) as io,
            tc.tile_pool(name="wk", bufs=1) as wk,
            tc.tile_pool(name="psum", bufs=1, space="PSUM") as psum,
        ):
            yt = io.tile([128, 512], bf16)
            yslabt = io.tile([128, 64], bf16)
            pen = io.tile([128, KH], bf16)
            ypos = io.tile([128, 8, D], bf16)
            ys2rep = io.tile([128, 4, D], bf16)
            permt = io.tile([128, 128], bf16)
            ha = io.tile([128, KH], bf16)
            hb = io.tile([128, KH], bf16)

            # DMA queues (completion latency ~4us; firsts matter):
            nc.sync.dma_start(out=ypos[:], in_=ypos_d[:].rearrange("p (s d) -> p s d", s=8))
            nc.sync.dma_start(out=pen[:], in_=pen_d[:])
            nc.scalar.dma_start(out=ys2rep[:], in_=ys2rep_d[:].rearrange("p (s d) -> p s d", s=4))
            nc.scalar.dma_start(out=ha[:], in_=ha_d[:])
            nc.scalar.dma_start(out=hb[:], in_=hb_d[:])
            nc.gpsimd.dma_start(out=yt[:], in_=yt_d[:])
            nc.gpsimd.dma_start(out=yslabt[:], in_=yslabt_d[:])
            nc.gpsimd.dma_start(out=permt[:], in_=permt_d[:])

            ones = wk.tile([128, 64], bf16)
            nc.gpsimd.memset(ones[:], 1.0)
            zeros = wk.tile([128, KH], bf16)
            nc.gpsimd.memset(zeros[:], 0.0)

            # ---------------- c-path: praw[p,s] = ||ypos - y||^2 (VectorE)
            diff = wk.tile([128, 8, D], bf16)
            nc.vector.tensor_sub(diff[:, 0:4, :], ypos[:, 0:4, :], ys2rep[:])
            nc.vector.tensor_sub(diff[:, 4:8, :], ypos[:, 4:8, :], ys2rep[:])
            dsq = wk.tile([128, 8, D], bf16)
            nc.vector.tensor_mul(dsq[:], diff[:], diff[:])
            praw = wk.tile([128, 8], f32)

            # ---------------- E path
            sqi = wk.tile([128, 1], f32)
            scr_q = wk.tile([128, D], bf16)
            nc.scalar.activation(
                out=scr_q[:], in_=ys2rep[:, 0, :],
                func=mybir.ActivationFunctionType.Square, accum_out=sqi[:],
            )
            ytsq = wk.tile([128, 512], bf16)
            nc.scalar.activation(
                out=ytsq[:], in_=yt[:], func=mybir.ActivationFunctionType.Square
            )
            n2yst = wk.tile([128, 64], bf16)
            nc.vector.tensor_scalar_mul(n2yst[:], yslabt[:], -2.0)

            psum_e = psum.tile([128, KH], f32)
            for h in (0, 1):
                pslice = psum_e[h * 64 : (h + 1) * 64, :]
                ksl = slice(h * KH, (h + 1) * KH)
                tp = (0, h * 64)
                nc.tensor.matmul(
                    pslice, n2yst[:], yt[:, ksl], start=True, stop=False,
                    tile_position=tp,
                )
                nc.tensor.matmul(
                    pslice, ones[:], ytsq[:, ksl], start=False, stop=True,
                    tile_position=tp,
                )

            # E = psum_e + sqi + pen  (one fused stt on VectorE), then praw
            e_sb = wk.tile([128, KH], bf16)
            nc.vector.scalar_tensor_tensor(
                out=e_sb[:], in0=psum_e[:], scalar=sqi[:], in1=pen[:],
                op0=mybir.AluOpType.add, op1=mybir.AluOpType.add,
            )
            praw_bf = wk.tile([128, 8], bf16)
            psum_p = psum.tile([128, 8], f32)
            nc.vector.reduce_sum(praw[:, 0:4], dsq[:, 0:4, :], axis=mybir.AxisListType.X)
            nc.vector.tensor_copy(praw_bf[:, 0:4], praw[:, 0:4])
            nc.tensor.matmul(psum_p[:, 0:4], permt[:], praw_bf[:, 0:4], start=True, stop=True)
            nc.vector.reduce_sum(praw[:, 4:8], dsq[:, 4:8, :], axis=mybir.AxisListType.X)
            nc.vector.tensor_copy(praw_bf[:, 4:8], praw[:, 4:8])
            nc.tensor.matmul(psum_p[:, 4:8], permt[:], praw_bf[:, 4:8], start=True, stop=True)

            # ---------------- hier (early, on ScalarE + GpSimd)
            dh = wk.tile([128, KH], bf16)
            nc.gpsimd.tensor_sub(dh[:], ha[:], hb[:])
            outa = wk.tile([128, 9], f32)
            outd = wk.tile([128, 9], f32)
            rs_ps = psum.tile([128, 9], f32)
            scr_h = wk.tile([128, KH], bf16)
            nc.scalar.activation(
                out=scr_h[:], in_=dh[:],
                func=mybir.ActivationFunctionType.Square,
                accum_out=rs_ps[:, 8:9],
            )

            # ---------------- 16 hinge row-sum instructions
            scr_a = wk.tile([128, KH], bf16)
            scr_d = wk.tile([128, KH], bf16)

            def bias(s):
                return praw[:, s : s + 1] if s < 8 else psum_p[:, s - 8 : s - 7]

            for ci, s in enumerate(ACT_SLOTS):
                nc.scalar.activation(
                    out=scr_a[:], in_=e_sb[:],
                    func=mybir.ActivationFunctionType.Relu,
                    bias=bias(s), scale=-1.0,
                    accum_out=rs_ps[:, ci : ci + 1],
                )
            dve_order = [8, 9, 10, 11, 12, 13, 14, 15]
            for s in dve_order:
                ci = DVE_SLOTS.index(s)
                nc.vector.scalar_tensor_tensor(
                    out=scr_d[:], in0=e_sb[:], scalar=bias(s), in1=zeros[:],
                    op0=mybir.AluOpType.subtract, op1=mybir.AluOpType.min,
                    accum_out=outd[:, ci : ci + 1],
                )
            nc.scalar.copy(outa[:], rs_ps[:])
            nc.scalar.dma_start(out=outa_d[:], in_=outa[:])
            nc.sync.dma_start(out=outd_d[:], in_=outd[:])

    nc.finalize()
    return nc


def _get_nc():
    if "nc" not in _NC_CACHE:
        _NC_CACHE["nc"] = _build_nc()
    return _NC_CACHE["nc"]


def _install_ntff_hook():
    """Provide antenv.axon_hooks if the image lacks it, so trace=True can
    capture NTFF profiles through the axon PJRT .so."""
    import sys
    import types

    try:
        from antenv.axon_hooks import get_axon_ntff_profile_hook  # noqa: F401

        return
    except ImportError:
        pass
    try:
        import antenv
        from trn_agent_boot.trn_boot import _ntff_profile_via_ctypes
    except ImportError:
        return
    mod = types.ModuleType("antenv.axon_hooks")
    state = {"h": None}
    mod.set_axon_ntff_profile_hook = lambda h: state.__setitem__("h", h)
    mod.get_axon_ntff_profile_hook = lambda: state["h"]
    sys.modules["antenv.axon_hooks"] = mod
    antenv.axon_hooks = mod
    try:
        hook = _ntff_profile_via_ctypes("/opt/axon/libaxon_pjrt.so")
    except OSError:
        hook = None
    mod.set_axon_ntff_profile_hook(hook)


def kernel(wid_pos_mu, ken_pos_mu, lrg_pos_mu, sml_pos_mu, yad_pos, x):
    global LAST_EXEC_TIME_NS
    wid = np.asarray(wid_pos_mu, dtype=np.float32)
    ken = np.asarray(ken_pos_mu, dtype=np.float32)
    lrg = np.asarray(lrg_pos_mu, dtype=np.float32)
    sml = np.asarray(sml_pos_mu, dtype=np.float32)
    yad = np.asarray(yad_pos, dtype=np.float32)
    xi = np.asarray(x).astype(np.int64)

    in_maps, weights = _host_pack(yad, wid, ken, lrg, sml, xi)

    from concourse.bass_utils import run_bass_kernel_spmd

    nc = _get_nc()
    trace = bool(int(os.environ.get("KERNEL_TRACE", "0")))
    if trace:
        _install_ntff_hook()
    res = run_bass_kernel_spmd(
        nc, in_maps, core_ids=list(range(NCORES)), trace=trace,
        tmpdir=os.environ.get("KERNEL_TMPDIR") or None,
    )
    LAST_EXEC_TIME_NS = res.exec_time_ns

    return np.float32(_gather_host(res.results, weights))


if __name__ == "__main__":
    # Smoke test of the numpy model against a direct dense recompute.
    rng = np.random.default_rng(0)
    yad = rng.standard_normal((N, D)).astype(np.float32)
    wid = rng.standard_normal((N, D)).astype(np.float32)
    ken = rng.standard_normal((N, D)).astype(np.float32)
    lrg = rng.standard_normal((N, D)).astype(np.float32)
    sml = rng.standard_normal((N, D)).astype(np.float32)
    x = rng.integers(0, N, size=(N, K)).astype(np.int64)

    def dense_ref(wid, ken, lrg, sml, yad, x):
        loss = (
            ((wid - ken) ** 2).sum()
            + ((wid - lrg) ** 2).sum()
            + ((lrg - sml) ** 2).sum()
            + ((sml - yad) ** 2).sum()
        )
        m = np.zeros((N, N), bool)
        m[np.arange(N)[:, None], x] = True
        eye = np.eye(N, dtype=bool)
        pos = m & ~eye
        neg = (~m) & ~eye
        sq = (yad * yad).sum(-1)
        gram = yad @ yad.T
        d2 = sq[:, None] + sq[None, :] - 2.0 * gram
        t = d2[:, :, None] - d2[:, None, :] + ALPHA
        valid = pos[:, :, None] & neg[:, None, :]
        return loss + np.where(valid, np.maximum(t, 0.0), 0.0).sum()

    ref = dense_ref(
        wid.astype(np.float64), ken.astype(np.float64), lrg.astype(np.float64),
        sml.astype(np.float64), yad.astype(np.float64), x,
    )
    got = model_numpy(_host_pack(yad, wid, ken, lrg, sml, x))
    print("dense ref:", ref)
    print("model    :", got)
    print("rel err  :", abs(got - ref) / abs(ref))
